# revision 16
# baseline (speedup 1.0000x reference)
"""Sparse (diffusion block-causal) GQA attention on 8 Trainium2 NeuronCores.

Contract: kernel(**inputs) takes the FULL inputs
    q [2048, 4096] f32, k [2048, 1024] f32, v [2048, 1024] f32,
    block_mask [2048, 2048] bool
and returns the FULL output [2048, 4096] f32.

Sharding: tensor-parallel over KV heads. Core c owns KV head c and its 4
GQA query heads (output columns [512c, 512c+512)). block_mask handled by
compiling a per-mask-pattern schedule (full / empty / partial 128x512
tiles); partial tiles get an additive -1e30 mask folded in via an extra
accumulating identity-matmul. No inter-core communication.

Device algorithm per core (S^T layout, no on-device transposes):
  for each q-head h (4) and q-chunk J (512 wide):
    for each active k-tile j (128 wide):
      S^T[kj, qJ] = kT_j contracted with qT chunk     (PE, float32r)
      (+ -1e30 mask add via bf16 identity matmul on partial tiles,
       with fully-masked q-prefixes pruned from every matmul)
    exp via ACT (scale = 1/sqrt(128) folded in) -> fp16 SBUF
    O^T[d, qJ] += V_j^T @ expS                        (PE, PSUM accum)
    softmax denominators: fp16 accumulation (full tiles on DVE at the
    2x perf mode, partial tiles on the otherwise-idle Pool engine),
    then one fp16 ones-matmul per chunk reduces partitions (PE)
  per chunk: reciprocal on DVE (custom approx op), partition-broadcast
  via a DRAM-bounce DMA, one DVE multiply (PSUM x SBUF) normalizes,
  DMA out. Cross-chunk software pipelining as before.

Host does the layout transposes during shard/gather (not part of HW time).
"""

import os
import sys

import numpy as np

for _p in ("/opt/trn_rl_repo",):
    if _p not in sys.path and os.path.isdir(_p):
        sys.path.insert(0, _p)

S = 2048
H = 32
HKV = 8
G = H // HKV  # 4 query heads per kv head
D = 128
NCORES = 8
SCALE = float(D) ** -0.5
CHUNK = 512  # q columns per S^T matmul (fp32 moving-operand max)
KT = 128  # k rows per tile (PE partition dim)
GROUP_KT = 2  # k-tiles exp'd per ACT call (2 PSUM banks)
NEG = -1.0e30

# Engine for partial-tile denominator accumulation: "pool" offloads to the
# idle GPSIMD engine, "dve" keeps everything on DVE.
PARTIAL_ACC_ENGINE = "dve"
COMBINE_ENGINE = "dve"  # engine for the per-chunk acc_p + acc_f combine
PS_BUFS = 3  # score PSUM tile buffers (each GROUP_KT banks)
PO_BUFS = 2
ES_BUFS = 5
OTN_BUFS = 3
PB_BUFS = 3
ACC_BUFS = 3
CHUNK_ORDER = "desc"  # "desc" per head, or "byJ" (all heads J=3, then J=2, ...)

NJ = S // CHUNK  # q chunks
NK = S // KT  # k tiles

_program_cache = {}
last_exec_time_ns = None
last_results = None


def _schedule_from_mask(bm):
    """Classify each (q-chunk J, k-tile j) as full / empty / partial.

    Returns (cache_key, sched, patterns): sched[J] is a list of
    (j, pattern_idx_or_None); patterns is a list of additive-mask arrays
    [KT, CHUNK] f32 (0 where attending, NEG where masked), k-major layout
    to match the S^T tile orientation.
    """
    sched = []
    patterns = []
    pat_idx = {}
    pat_q0 = {}
    for J in range(NJ):
        rows = bm[J * CHUNK : (J + 1) * CHUNK]  # [CHUNK q, S k]
        row = []
        for j in range(NK):
            sub = rows[:, j * KT : (j + 1) * KT]  # [q, k]
            if sub.all():
                row.append((j, None, 0))
            elif not sub.any():
                continue
            else:
                key = sub.tobytes()
                if key not in pat_idx:
                    pat_idx[key] = len(patterns)
                    patterns.append(
                        np.where(sub.T, np.float32(0.0), np.float32(NEG))
                    )
                    # first q row with any active cell: columns before it
                    # are fully masked and can be skipped entirely
                    pat_q0[pat_idx[key]] = int(np.argmax(sub.any(axis=1)))
                row.append((j, pat_idx[key], pat_q0[pat_idx[key]]))
        assert row, f"q-chunk {J} attends to nothing"
        # The first tile's start=True must cover the full q range of the
        # PV/sums accumulators.
        if row[0][2] != 0:
            row[0] = (row[0][0], row[0][1], 0)
        sched.append(row)
    cache_key = tuple(
        tuple(r for r in row) for row in sched
    ), tuple(p.tobytes() for p in patterns)
    return hash(cache_key), sched, patterns


def _build_program(sched, patterns, reps=1):
    import contextlib

    import concourse.bacc as bacc
    import concourse.tile as tile
    from concourse import mybir

    f32 = mybir.dt.float32
    f32r = mybir.dt.float32r
    f16 = mybir.dt.float16
    bf16 = mybir.dt.bfloat16
    EXP = mybir.ActivationFunctionType.Exp
    LN = mybir.ActivationFunctionType.Ln

    nc = bacc.Bacc(
        "TRN2", target_bir_lowering=False, debug=False, num_devices=NCORES
    )

    qT = nc.dram_tensor("qT", [G, D, S], f32r, kind="ExternalInput").ap()
    kT = nc.dram_tensor("kT", [D, S], f32r, kind="ExternalInput").ap()
    v = nc.dram_tensor("v", [S, D], f16, kind="ExternalInput").ap()
    n_pat = max(1, len(patterns))
    pmask = nc.dram_tensor(
        "pmask", [n_pat, KT, CHUNK], bf16, kind="ExternalInput"
    ).ap()
    ident = nc.dram_tensor("ident", [D, D], bf16, kind="ExternalInput").ap()
    oT = nc.dram_tensor("oT", [G, D, S], f32, kind="ExternalOutput").ap()

    n_chunks = G * NJ  # 16 (head, chunk) pairs

    with tile.TileContext(nc) as tc:
        with (
            tc.tile_pool(name="singles", bufs=1) as singles,
            tc.tile_pool(name="ps", bufs=PS_BUFS, space="PSUM") as ps_pool,
            tc.tile_pool(name="po", bufs=PO_BUFS, space="PSUM") as po_pool,
            tc.tile_pool(name="es", bufs=ES_BUFS) as es_pool,
            tc.tile_pool(name="otn", bufs=OTN_BUFS) as otn_pool,
            tc.tile_pool(name="rows", bufs=4) as rows_pool,
            tc.tile_pool(name="pbp", bufs=PB_BUFS) as pb_pool,
            tc.tile_pool(name="accp", bufs=ACC_BUFS) as acc_pool,
        ):
            # Resident inputs. DMA order matters for the startup critical
            # path: tiny constants, then the first head/chunk's operands in
            # 512-column pieces, then the rest.
            qT_sb = singles.tile([D, G * S], f32r)
            kT_sb = singles.tile([D, S], f32r)
            v_sb = singles.tile([KT, NK * D], f16)
            pm_sb = singles.tile([KT, n_pat * CHUNK], bf16)
            id_sb = singles.tile([D, D], bf16)

            # Few, large input DMAs (HWDGE issue costs ~0.6us per DMA):
            # kT chunk0 + h0's first q chunk first, bulk after.
            nc.sync.dma_start(out=kT_sb[:, 0:KT], in_=kT[:, 0:KT])
            nc.sync.dma_start(
                out=qT_sb[:, 3 * CHUNK : 4 * CHUNK],
                in_=qT[0][:, 3 * CHUNK : 4 * CHUNK],
            )
            nc.sync.dma_start(out=kT_sb[:, KT:CHUNK], in_=kT[:, KT:CHUNK])
            nc.sync.dma_start(
                out=kT_sb[:, CHUNK:], in_=kT[:, CHUNK:]
            )
            nc.sync.dma_start(
                out=v_sb.rearrange("p (t d) -> p t d", d=D),
                in_=v.rearrange("(t p) d -> p t d", p=KT),
            )
            nc.sync.dma_start(
                out=pm_sb.rearrange("p (n c) -> p n c", c=CHUNK),
                in_=pmask.rearrange("n p c -> p n c"),
            )
            nc.sync.dma_start(out=id_sb, in_=ident)
            nc.sync.dma_start(
                out=qT_sb[:, 0 : 3 * CHUNK], in_=qT[0][:, 0 : 3 * CHUNK]
            )
            nc.sync.dma_start(
                out=qT_sb[:, S:].rearrange("p (h s) -> p h s", s=S),
                in_=qT[1:].rearrange("h p s -> p h s"),
            )

            rep_ctx = (
                tc.For_i(0, reps, 1) if reps > 1 else contextlib.nullcontext()
            )

            def emit_epilogue(ctx):
                # Normalize and store chunk ctx: runs one exp-group after
                # the chunk's last PV matmul (cross-chunk pipelined).
                # All-on-chip chain: the Pool all-reduce already left the
                # broadcast denominators in ctx["lall"]; DVE reciprocal +
                # multiply free the po PSUM bank fast.
                h, J, po = ctx["h"], ctx["J"], ctx["po"]
                otn = otn_pool.tile([D, CHUNK], f32)
                rb = pb_pool.tile([D, CHUNK], f32, tag="rb")
                nc.vector.reciprocal_approx_fast(rb, ctx["lall"])
                nc.vector.tensor_mul(otn, po, rb)
                nc.sync.dma_start(
                    out=oT[h][:, J * CHUNK : (J + 1) * CHUNK], in_=otn
                )

            def emit_pv(grp_es, grp, ctx):
                po = ctx["po"]
                for t, (j, pidx, q0) in enumerate(grp):
                    sl = grp_es[:, t * CHUNK + q0 : (t + 1) * CHUNK]
                    first = ctx["pv_done"] == 0
                    last = ctx["pv_done"] == ctx["nk"] - 1
                    nc.tensor.matmul(
                        po[:, q0:],
                        lhsT=v_sb[:, j * D : (j + 1) * D],
                        rhs=sl,
                        start=first,
                        stop=last,
                    )
                    # fp16 denominator accumulation; masked cells of partial
                    # tiles are exact zeros after exp(x - 1e30), so every
                    # tile can join.  Two independent accumulator chains:
                    # partial tiles on the otherwise-idle Pool engine,
                    # full tiles on DVE (2x fp16 mode) -- neither serializes
                    # on the other, and each finishes right after its last
                    # tile's exp.
                    if pidx is not None and PARTIAL_ACC_ENGINE == "pool":
                        if ctx["acc_p"] is None:
                            ctx["acc_p"] = acc_pool.tile(
                                [KT, CHUNK], f16, tag="acc_p", name="acc_p"
                            )
                            assert q0 == 0
                            nc.gpsimd.tensor_copy(ctx["acc_p"], sl)
                        else:
                            nc.gpsimd.tensor_add(
                                ctx["acc_p"][:, q0:], ctx["acc_p"][:, q0:], sl
                            )
                    else:
                        if ctx["acc"] is None:
                            ctx["acc"] = acc_pool.tile(
                                [KT, CHUNK], f16, tag="acc_f", name="acc"
                            )
                            if q0 != 0:
                                nc.vector.memset(ctx["acc"][:, :q0], 0.0)
                            nc.vector.tensor_copy(ctx["acc"][:, q0:], sl)
                        else:
                            nc.vector.tensor_add(
                                ctx["acc"][:, q0:], ctx["acc"][:, q0:], sl
                            )
                    ctx["pv_done"] += 1
                    if ctx["pv_done"] == ctx["nk"]:
                        # Combine the two fp16 chains (Pool), then a Pool
                        # all-reduce leaves the summed denominators
                        # broadcast across all partitions -- no PSUM, no PE.
                        from concourse import bass_isa

                        a = ctx["acc_p"] if ctx["acc_p"] is not None else ctx["acc"]
                        if ctx["acc_p"] is not None and ctx["acc"] is not None:
                            eng = (
                                nc.gpsimd
                                if COMBINE_ENGINE == "pool"
                                else nc.vector
                            )
                            eng.tensor_add(a, a, ctx["acc"])
                        lall = pb_pool.tile([D, CHUNK], f32, tag="lall")
                        nc.gpsimd.partition_all_reduce(
                            lall, a, channels=D,
                            reduce_op=bass_isa.ReduceOp.add,
                        )
                        ctx["lall"] = lall
                if ctx["pv_done"] == ctx["nk"]:
                    emit_epilogue(ctx)

            with rep_ctx:
                prev = None  # (es_tile, group, ctx) awaiting PV emission
                cidx = 0
                if CHUNK_ORDER == "byJ":
                    hj_order = [
                        (h, J)
                        for J in sorted(range(NJ), reverse=True)
                        for h in range(G)
                    ]
                else:
                    hj_order = [
                        (h, J)
                        for h in range(G)
                        for J in (
                            sorted(range(NJ), reverse=True)
                            if NJ == 4
                            else range(NJ)
                        )
                    ]
                if True:
                    for h, J in hj_order:
                        tiles = sched[J]
                        # Partial (diagonal) tiles first: their Pool-side
                        # accumulator chain starts early, and the chunk's
                        # denominator is ready right after the last full
                        # tile's exp.  Full tiles exp in GROUP_KT-wide
                        # PSUM groups; partial tiles get their own unit so
                        # the exp can skip the pruned prefix.
                        full_t = [t for t in tiles if t[1] is None]
                        part_t = [t for t in tiles if t[1] is not None]
                        ordered = part_t + full_t
                        if ordered[0][2] != 0:
                            ordered[0] = (ordered[0][0], ordered[0][1], 0)
                        groups = [[t] for t in ordered[: len(part_t)]] + [
                            full_t[g : g + GROUP_KT]
                            for g in range(0, len(full_t), GROUP_KT)
                        ]
                        ctx = {
                            "cidx": cidx,
                            "h": h,
                            "J": J,
                            "po": po_pool.tile([D, CHUNK], f32, tag="po", name="po"),
                            "pv_done": 0,
                            "nk": len(ordered),
                            "acc": None,
                            "acc_p": None,
                        }
                        rhs_q = qT_sb[
                            :, h * S + J * CHUNK : h * S + (J + 1) * CHUNK
                        ]
                        for grp in groups:
                            gw = len(grp) * CHUNK
                            lo = grp[0][2]  # >0 only for partial singleton
                            ps = ps_pool.tile(
                                [KT, len(grp) * CHUNK], f32, tag="ps"
                            )
                            for t, (j, pidx, q0) in enumerate(grp):
                                out_sl = ps[
                                    :, t * CHUNK + q0 : (t + 1) * CHUNK
                                ]
                                nc.tensor.matmul(
                                    out_sl,
                                    lhsT=kT_sb[:, j * KT : (j + 1) * KT],
                                    rhs=rhs_q[:, q0:],
                                    start=True,
                                    stop=(pidx is None),
                                )
                                if pidx is not None:
                                    nc.tensor.matmul(
                                        out_sl,
                                        lhsT=id_sb,
                                        rhs=pm_sb[
                                            :,
                                            pidx * CHUNK + q0 : (pidx + 1)
                                            * CHUNK,
                                        ],
                                        start=False,
                                        stop=True,
                                    )
                            if prev is not None:
                                emit_pv(*prev)
                                prev = None
                            es = es_pool.tile(
                                [KT, len(grp) * CHUNK], f16, tag="es"
                            )
                            nc.scalar.activation(
                                es[:, lo:gw], ps[:, lo:gw], EXP, scale=SCALE
                            )
                            prev = (es, grp, ctx)
                        cidx += 1
                emit_pv(*prev)
                prev = None

    # Pin the ACT table set to the one containing both Exp and Ln so the
    # table-load pass emits exactly one load.
    import concourse.bacc as bacc_mod

    orig_tables = bacc_mod.get_activation_tables

    def _only_ln_exp_set(arch):
        return {
            name: (fns if name == "natural_log_exp_and_others" else set())
            for name, fns in orig_tables(arch).items()
        }

    bacc_mod.get_activation_tables = _only_ln_exp_set
    try:
        nc.compile()
    finally:
        bacc_mod.get_activation_tables = orig_tables
    return nc


def _get_program(bm):
    key, sched, patterns = _schedule_from_mask(bm)
    if key not in _program_cache:
        _program_cache[key] = _build_program(sched, patterns)
    return _program_cache[key], patterns


def _shard_inputs(q, k, v, patterns):
    import ml_dtypes

    bf16 = ml_dtypes.bfloat16
    n_pat = max(1, len(patterns))
    if patterns:
        pm = np.ascontiguousarray(np.stack(patterns).astype(bf16))
    else:
        pm = np.zeros((n_pat, KT, CHUNK), bf16)
    ident = np.eye(D, dtype=bf16)

    q5 = q.reshape(S, HKV, G, D)
    k4 = k.reshape(S, HKV, D)
    v4 = v.reshape(S, HKV, D)
    in_maps = []
    for c in range(NCORES):
        qTc = np.ascontiguousarray(q5[:, c].transpose(1, 2, 0))  # [G, D, S]
        kTc = np.ascontiguousarray(k4[:, c].T)  # [D, S]
        vc = np.ascontiguousarray(v4[:, c].astype(np.float16))  # [S, D]
        in_maps.append(
            {
                "qT": qTc,
                "kT": kTc,
                "v": vc,
                "pmask": pm,
                "ident": ident,
            }
        )
    return in_maps


def kernel(q, k, v, block_mask):
    global last_exec_time_ns, last_results
    q = np.ascontiguousarray(np.asarray(q, dtype=np.float32))
    k = np.ascontiguousarray(np.asarray(k, dtype=np.float32))
    v = np.ascontiguousarray(np.asarray(v, dtype=np.float32))
    bm = np.ascontiguousarray(np.asarray(block_mask)).astype(bool)

    nc, patterns = _get_program(bm)
    _, _, patterns = _schedule_from_mask(bm)
    in_maps = _shard_inputs(q, k, v, patterns)

    from concourse.bass_utils import run_bass_kernel_spmd

    res = run_bass_kernel_spmd(nc, in_maps, list(range(NCORES)), trace=False)
    last_exec_time_ns = res.exec_time_ns
    last_results = res

    out = np.empty((S, H * D), np.float32)
    for c in range(NCORES):
        oTc = res.results[c]["oT"]  # [G, D, S]
        out[:, c * G * D : (c + 1) * G * D] = (
            oTc.transpose(2, 0, 1).reshape(S, G * D)
        )
    return out


# revision 17
# speedup vs baseline: 1.1226x; 1.1226x over previous
"""Sparse (diffusion block-causal) GQA attention on 8 Trainium2 NeuronCores.

Contract: kernel(**inputs) takes the FULL inputs
    q [2048, 4096] f32, k [2048, 1024] f32, v [2048, 1024] f32,
    block_mask [2048, 2048] bool
and returns the FULL output [2048, 4096] f32.

Sharding: tensor-parallel over KV heads. Core c owns KV head c and its 4
GQA query heads (output columns [512c, 512c+512)). block_mask handled by
compiling a per-mask-pattern schedule (full / empty / partial 128x512
tiles); partial tiles get an additive -1e30 mask folded in via an extra
accumulating identity-matmul. No inter-core communication.

Device algorithm per core (S^T layout, no on-device transposes):
  for each q-head h (4) and q-chunk J (512 wide):
    for each active k-tile j (128 wide):
      S^T[kj, qJ] = kT_j contracted with qT chunk     (PE, float32r)
      (+ -1e30 mask add via bf16 identity matmul on partial tiles,
       with fully-masked q-prefixes pruned from every matmul)
    exp via ACT (scale = 1/sqrt(128) folded in) -> fp16 SBUF
    O^T[d, qJ] += V_j^T @ expS                        (PE, PSUM accum)
    softmax denominators: fp16 accumulation (full tiles on DVE at the
    2x perf mode, partial tiles on the otherwise-idle Pool engine),
    then one fp16 ones-matmul per chunk reduces partitions (PE)
  per chunk: reciprocal on DVE (custom approx op), partition-broadcast
  via a DRAM-bounce DMA, one DVE multiply (PSUM x SBUF) normalizes,
  DMA out. Cross-chunk software pipelining as before.

Host does the layout transposes during shard/gather (not part of HW time).
"""

import os
import sys

import numpy as np

for _p in ("/opt/trn_rl_repo",):
    if _p not in sys.path and os.path.isdir(_p):
        sys.path.insert(0, _p)

S = 2048
H = 32
HKV = 8
G = H // HKV  # 4 query heads per kv head
D = 128
NCORES = 8
SCALE = float(D) ** -0.5
CHUNK = 512  # q columns per S^T matmul (fp32 moving-operand max)
KT = 128  # k rows per tile (PE partition dim)
GROUP_KT = 2  # k-tiles exp'd per ACT call (2 PSUM banks)
NEG = -1.0e30

# Engine for partial-tile denominator accumulation: "pool" offloads to the
# idle GPSIMD engine, "dve" keeps everything on DVE.
PARTIAL_ACC_ENGINE = "dve"
COMBINE_ENGINE = "dve"  # engine for the per-chunk acc_p + acc_f combine
PS_BUFS = 2  # score PSUM tile buffers (each GROUP_KT banks)
PO_BUFS = 3
ES_BUFS = 5
OTN_BUFS = 3
PB_BUFS = 3
ACC_BUFS = 3
CHUNK_ORDER = "desc"  # "desc" per head, or "byJ" (all heads J=3, then J=2, ...)

NJ = S // CHUNK  # q chunks
NK = S // KT  # k tiles

_program_cache = {}
last_exec_time_ns = None
last_results = None


def _schedule_from_mask(bm):
    """Classify each (q-chunk J, k-tile j) as full / empty / partial.

    Returns (cache_key, sched, patterns): sched[J] is a list of
    (j, pattern_idx_or_None); patterns is a list of additive-mask arrays
    [KT, CHUNK] f32 (0 where attending, NEG where masked), k-major layout
    to match the S^T tile orientation.
    """
    sched = []
    patterns = []
    pat_idx = {}
    pat_q0 = {}
    for J in range(NJ):
        rows = bm[J * CHUNK : (J + 1) * CHUNK]  # [CHUNK q, S k]
        row = []
        for j in range(NK):
            sub = rows[:, j * KT : (j + 1) * KT]  # [q, k]
            if sub.all():
                row.append((j, None, 0))
            elif not sub.any():
                continue
            else:
                key = sub.tobytes()
                if key not in pat_idx:
                    pat_idx[key] = len(patterns)
                    patterns.append(
                        np.where(sub.T, np.float32(0.0), np.float32(NEG))
                    )
                    # first q row with any active cell: columns before it
                    # are fully masked and can be skipped entirely
                    pat_q0[pat_idx[key]] = int(np.argmax(sub.any(axis=1)))
                row.append((j, pat_idx[key], pat_q0[pat_idx[key]]))
        assert row, f"q-chunk {J} attends to nothing"
        # The first tile's start=True must cover the full q range of the
        # PV/sums accumulators.
        if row[0][2] != 0:
            row[0] = (row[0][0], row[0][1], 0)
        sched.append(row)
    cache_key = tuple(
        tuple(r for r in row) for row in sched
    ), tuple(p.tobytes() for p in patterns)
    return hash(cache_key), sched, patterns


def _build_program(sched, patterns, reps=1):
    import contextlib

    import concourse.bacc as bacc
    import concourse.tile as tile
    from concourse import mybir

    f32 = mybir.dt.float32
    f32r = mybir.dt.float32r
    f16 = mybir.dt.float16
    bf16 = mybir.dt.bfloat16
    EXP = mybir.ActivationFunctionType.Exp
    LN = mybir.ActivationFunctionType.Ln

    nc = bacc.Bacc(
        "TRN2", target_bir_lowering=False, debug=False, num_devices=NCORES
    )

    qT = nc.dram_tensor("qT", [G, D, S], f32r, kind="ExternalInput").ap()
    kT = nc.dram_tensor("kT", [D, S], f32r, kind="ExternalInput").ap()
    v = nc.dram_tensor("v", [S, D], f16, kind="ExternalInput").ap()
    n_pat = max(1, len(patterns))
    pmask = nc.dram_tensor(
        "pmask", [n_pat, KT, CHUNK], bf16, kind="ExternalInput"
    ).ap()
    ident = nc.dram_tensor("ident", [D, D], bf16, kind="ExternalInput").ap()
    onesc = nc.dram_tensor("onesc", [KT, 1], f16, kind="ExternalInput").ap()
    oT = nc.dram_tensor("oT", [G, D, S], f32, kind="ExternalOutput").ap()

    n_chunks = G * NJ  # 16 (head, chunk) pairs

    with tile.TileContext(nc) as tc:
        with (
            tc.tile_pool(name="singles", bufs=1) as singles,
            tc.tile_pool(name="ps", bufs=PS_BUFS, space="PSUM") as ps_pool,
            tc.tile_pool(name="po", bufs=PO_BUFS, space="PSUM") as po_pool,
            tc.tile_pool(name="nrm", bufs=1, space="PSUM") as nrm_pool,
            tc.tile_pool(name="es", bufs=ES_BUFS) as es_pool,
            tc.tile_pool(name="otn", bufs=OTN_BUFS) as otn_pool,
            tc.tile_pool(name="rows", bufs=4) as rows_pool,
            tc.tile_pool(name="pbp", bufs=PB_BUFS) as pb_pool,
            tc.tile_pool(name="accp", bufs=ACC_BUFS) as acc_pool,
        ):
            # Resident inputs. DMA order matters for the startup critical
            # path: tiny constants, then the first head/chunk's operands in
            # 512-column pieces, then the rest.
            qT_sb = singles.tile([D, G * S], f32r)
            kT_sb = singles.tile([D, S], f32r)
            v_sb = singles.tile([KT, NK * D], f16)
            pm_sb = singles.tile([KT, n_pat * CHUNK], bf16)
            id_sb = singles.tile([D, D], bf16)
            ones_col = singles.tile([KT, 1], f16)

            # Few, large input DMAs (HWDGE issue costs ~0.6us per DMA):
            # kT chunk0 + h0's first q chunk first, bulk after.
            nc.sync.dma_start(out=kT_sb[:, 0:KT], in_=kT[:, 0:KT])
            nc.sync.dma_start(
                out=qT_sb[:, 3 * CHUNK : 4 * CHUNK],
                in_=qT[0][:, 3 * CHUNK : 4 * CHUNK],
            )
            nc.sync.dma_start(out=kT_sb[:, KT:CHUNK], in_=kT[:, KT:CHUNK])
            nc.sync.dma_start(
                out=kT_sb[:, CHUNK:], in_=kT[:, CHUNK:]
            )
            nc.sync.dma_start(
                out=v_sb.rearrange("p (t d) -> p t d", d=D),
                in_=v.rearrange("(t p) d -> p t d", p=KT),
            )
            nc.sync.dma_start(
                out=pm_sb.rearrange("p (n c) -> p n c", c=CHUNK),
                in_=pmask.rearrange("n p c -> p n c"),
            )
            nc.sync.dma_start(out=id_sb, in_=ident)
            nc.sync.dma_start(out=ones_col, in_=onesc)
            nc.sync.dma_start(
                out=qT_sb[:, 0 : 3 * CHUNK], in_=qT[0][:, 0 : 3 * CHUNK]
            )
            nc.sync.dma_start(
                out=qT_sb[:, S:].rearrange("p (h s) -> p h s", s=S),
                in_=qT[1:].rearrange("h p s -> p h s"),
            )

            rep_ctx = (
                tc.For_i(0, reps, 1) if reps > 1 else contextlib.nullcontext()
            )

            def emit_epilogue(ctx):
                # Normalize and store chunk ctx: runs one exp-group after
                # the chunk's last PV matmul (cross-chunk pipelined).
                # All-on-chip chain: the Pool all-reduce already left the
                # broadcast denominators in ctx["lall"]; DVE reciprocal +
                # multiply free the po PSUM bank fast.
                h, J, po = ctx["h"], ctx["J"], ctx["po"]
                otn = otn_pool.tile([D, CHUNK], f32)
                r_row = rows_pool.tile([1, CHUNK], f32, tag="rrow")
                nc.vector.reciprocal_approx_fast(r_row, ctx["psm"][:1, :])
                pb = pb_pool.tile([D, CHUNK], f32, tag="pb")
                nc.gpsimd.partition_broadcast(pb, r_row)
                nc.vector.tensor_mul(otn, po, pb)
                nc.sync.dma_start(
                    out=oT[h][:, J * CHUNK : (J + 1) * CHUNK], in_=otn
                )

            def emit_pv(grp_es, grp, ctx):
                po = ctx["po"]
                for t, (j, pidx, q0) in enumerate(grp):
                    sl = grp_es[:, t * CHUNK + q0 : (t + 1) * CHUNK]
                    first = ctx["pv_done"] == 0
                    last = ctx["pv_done"] == ctx["nk"] - 1
                    nc.tensor.matmul(
                        po[:, q0:],
                        lhsT=v_sb[:, j * D : (j + 1) * D],
                        rhs=sl,
                        start=first,
                        stop=last,
                    )
                    # fp16 denominator accumulation; masked cells of partial
                    # tiles are exact zeros after exp(x - 1e30), so every
                    # tile can join.  Two independent accumulator chains:
                    # partial tiles on the otherwise-idle Pool engine,
                    # full tiles on DVE (2x fp16 mode) -- neither serializes
                    # on the other, and each finishes right after its last
                    # tile's exp.
                    if pidx is not None and PARTIAL_ACC_ENGINE == "pool":
                        if ctx["acc_p"] is None:
                            ctx["acc_p"] = acc_pool.tile(
                                [KT, CHUNK], f16, tag="acc_p", name="acc_p"
                            )
                            assert q0 == 0
                            nc.gpsimd.tensor_copy(ctx["acc_p"], sl)
                        else:
                            nc.gpsimd.tensor_add(
                                ctx["acc_p"][:, q0:], ctx["acc_p"][:, q0:], sl
                            )
                    else:
                        if ctx["acc"] is None:
                            ctx["acc"] = acc_pool.tile(
                                [KT, CHUNK], f16, tag="acc_f", name="acc"
                            )
                            if q0 != 0:
                                nc.vector.memset(ctx["acc"][:, :q0], 0.0)
                            nc.vector.tensor_copy(ctx["acc"][:, q0:], sl)
                        else:
                            nc.vector.tensor_add(
                                ctx["acc"][:, q0:], ctx["acc"][:, q0:], sl
                            )
                    ctx["pv_done"] += 1
                    if ctx["pv_done"] == ctx["nk"]:
                        # Partition-reduce the fp16 sums: one ones-matmul
                        # per live accumulator chain (PSUM-accumulated).
                        psm = nrm_pool.tile([1, CHUNK], f32, tag="psm", name="psm")
                        chains = [
                            a
                            for a in (ctx["acc_p"], ctx["acc"])
                            if a is not None
                        ]
                        for i, a in enumerate(chains):
                            nc.tensor.matmul(
                                psm[:1, :],
                                lhsT=ones_col,
                                rhs=a,
                                start=(i == 0),
                                stop=(i == len(chains) - 1),
                            )
                        ctx["psm"] = psm
                if ctx["pv_done"] == ctx["nk"]:
                    emit_epilogue(ctx)

            with rep_ctx:
                prev = None  # (es_tile, group, ctx) awaiting PV emission
                cidx = 0
                if CHUNK_ORDER == "byJ":
                    hj_order = [
                        (h, J)
                        for J in sorted(range(NJ), reverse=True)
                        for h in range(G)
                    ]
                else:
                    hj_order = [
                        (h, J)
                        for h in range(G)
                        for J in (
                            sorted(range(NJ), reverse=True)
                            if NJ == 4
                            else range(NJ)
                        )
                    ]
                if True:
                    for h, J in hj_order:
                        tiles = sched[J]
                        # Partial (diagonal) tiles first: their Pool-side
                        # accumulator chain starts early, and the chunk's
                        # denominator is ready right after the last full
                        # tile's exp.  Full tiles exp in GROUP_KT-wide
                        # PSUM groups; partial tiles get their own unit so
                        # the exp can skip the pruned prefix.
                        full_t = [t for t in tiles if t[1] is None]
                        part_t = [t for t in tiles if t[1] is not None]
                        ordered = part_t + full_t
                        if ordered[0][2] != 0:
                            ordered[0] = (ordered[0][0], ordered[0][1], 0)
                        groups = [[t] for t in ordered[: len(part_t)]] + [
                            full_t[g : g + GROUP_KT]
                            for g in range(0, len(full_t), GROUP_KT)
                        ]
                        ctx = {
                            "cidx": cidx,
                            "h": h,
                            "J": J,
                            "po": po_pool.tile([D, CHUNK], f32, tag="po", name="po"),
                            "pv_done": 0,
                            "nk": len(ordered),
                            "acc": None,
                            "acc_p": None,
                        }
                        rhs_q = qT_sb[
                            :, h * S + J * CHUNK : h * S + (J + 1) * CHUNK
                        ]
                        for grp in groups:
                            gw = len(grp) * CHUNK
                            lo = grp[0][2]  # >0 only for partial singleton
                            ps = ps_pool.tile(
                                [KT, len(grp) * CHUNK], f32, tag="ps"
                            )
                            for t, (j, pidx, q0) in enumerate(grp):
                                out_sl = ps[
                                    :, t * CHUNK + q0 : (t + 1) * CHUNK
                                ]
                                nc.tensor.matmul(
                                    out_sl,
                                    lhsT=kT_sb[:, j * KT : (j + 1) * KT],
                                    rhs=rhs_q[:, q0:],
                                    start=True,
                                    stop=(pidx is None),
                                )
                                if pidx is not None:
                                    nc.tensor.matmul(
                                        out_sl,
                                        lhsT=id_sb,
                                        rhs=pm_sb[
                                            :,
                                            pidx * CHUNK + q0 : (pidx + 1)
                                            * CHUNK,
                                        ],
                                        start=False,
                                        stop=True,
                                    )
                            if prev is not None:
                                emit_pv(*prev)
                                prev = None
                            es = es_pool.tile(
                                [KT, len(grp) * CHUNK], f16, tag="es"
                            )
                            nc.scalar.activation(
                                es[:, lo:gw], ps[:, lo:gw], EXP, scale=SCALE
                            )
                            prev = (es, grp, ctx)
                        cidx += 1
                emit_pv(*prev)
                prev = None

    # Pin the ACT table set to the one containing both Exp and Ln so the
    # table-load pass emits exactly one load.
    import concourse.bacc as bacc_mod

    orig_tables = bacc_mod.get_activation_tables

    def _only_ln_exp_set(arch):
        return {
            name: (fns if name == "natural_log_exp_and_others" else set())
            for name, fns in orig_tables(arch).items()
        }

    bacc_mod.get_activation_tables = _only_ln_exp_set
    try:
        nc.compile()
    finally:
        bacc_mod.get_activation_tables = orig_tables
    return nc


def _get_program(bm):
    key, sched, patterns = _schedule_from_mask(bm)
    if key not in _program_cache:
        _program_cache[key] = _build_program(sched, patterns)
    return _program_cache[key], patterns


def _shard_inputs(q, k, v, patterns):
    import ml_dtypes

    bf16 = ml_dtypes.bfloat16
    n_pat = max(1, len(patterns))
    if patterns:
        pm = np.ascontiguousarray(np.stack(patterns).astype(bf16))
    else:
        pm = np.zeros((n_pat, KT, CHUNK), bf16)
    ident = np.eye(D, dtype=bf16)

    q5 = q.reshape(S, HKV, G, D)
    k4 = k.reshape(S, HKV, D)
    v4 = v.reshape(S, HKV, D)
    in_maps = []
    for c in range(NCORES):
        qTc = np.ascontiguousarray(q5[:, c].transpose(1, 2, 0))  # [G, D, S]
        kTc = np.ascontiguousarray(k4[:, c].T)  # [D, S]
        vc = np.ascontiguousarray(v4[:, c].astype(np.float16))  # [S, D]
        in_maps.append(
            {
                "qT": qTc,
                "kT": kTc,
                "v": vc,
                "pmask": pm,
                "ident": ident,
                "onesc": np.ones((KT, 1), np.float16),
            }
        )
    return in_maps


def kernel(q, k, v, block_mask):
    global last_exec_time_ns, last_results
    q = np.ascontiguousarray(np.asarray(q, dtype=np.float32))
    k = np.ascontiguousarray(np.asarray(k, dtype=np.float32))
    v = np.ascontiguousarray(np.asarray(v, dtype=np.float32))
    bm = np.ascontiguousarray(np.asarray(block_mask)).astype(bool)

    nc, patterns = _get_program(bm)
    _, _, patterns = _schedule_from_mask(bm)
    in_maps = _shard_inputs(q, k, v, patterns)

    from concourse.bass_utils import run_bass_kernel_spmd

    res = run_bass_kernel_spmd(nc, in_maps, list(range(NCORES)), trace=False)
    last_exec_time_ns = res.exec_time_ns
    last_results = res

    out = np.empty((S, H * D), np.float32)
    for c in range(NCORES):
        oTc = res.results[c]["oT"]  # [G, D, S]
        out[:, c * G * D : (c + 1) * G * D] = (
            oTc.transpose(2, 0, 1).reshape(S, G * D)
        )
    return out
